# revision 2
# baseline (speedup 1.0000x reference)
"""GAT layer (N=8192, F_IN=256, H=64 per head, K=8 heads) on 8 Trainium2 cores.

Strategy (row-sharding, fully data-parallel, no collectives):
  reference per head k:
    h   = features @ W[k]                      [N, H]
    wh1 = h @ a[k,:H]; wh2 = h @ a[k,H:]       [N]
    e   = leaky_relu(wh1[:,None] + wh2[None,:], 0.2)
    att = softmax(where(adj>0, e, -9e15), axis=1)
    out = elu(att @ h)

  Algebra: with s = wh1[i] + wh2[j],
    exp(lrelu(s)) = exp(0.2 s) * max(exp(0.8 s), 1)
                  = e1_{i} * e2_{j} * max(G1_i * G2_j, 1)
  where G1 = exp(0.8 wh1), G2 = exp(0.8 wh2), e1 = exp(0.2 wh1), e2 = exp(0.2 wh2).
  The row factor e1_i cancels in softmax.  The column factor e2_j is folded
  into the value matrix.  Masked entries: adj in {0,1} multiplies exactly.
    unnorm[i,j] = adj[i,j] * max(G1_i G2_j, 1) * e2_j   (up to the cancelled e1_i)
    out[i]      = elu( (unnorm @ h) / (unnorm @ 1) )
  On device the big [N/8, N] tensor per head is produced with only
  two DVE passes per tile (tensor_scalar mult+max fused, tensor_tensor mask)
  in bf16, and consumed by the PE with stationary [h*e2 | e2] per head.
  G1/G2/e2 come from tiny host matmuls (features @ (W[k] @ a[k])).

Per-core layout ([j,i]-transposed tiles so contraction j sits on partitions):
  adj_r  [128, 2, 64, 512] bf16 : adj_r[p,ib,g,i] = adj[r0+ib*512+i, g*128+p]
  g1b    [128, 8, 2, 512] bf16  : G1 row broadcast across partitions
  g2t    [128, 8, 64] f32       : g2t[p,k,g] = G2[g*128+p, k]  (per-partition scalars)
  e25t   [128, 64, 8] f32       : e25t[p,g,k] = e2[g*128+p, k]
  featT  [256, 8192] f32        : features.T (replicated; h computed on device)
  w_cat  [256, 512] f32         : all heads' W side by side
"""

import sys
import os

sys.path.insert(0, "/opt/trn_rl_repo")

import numpy as np
import ml_dtypes
from contextlib import ExitStack

import concourse.bass as bass
import concourse.tile as tile
from concourse import bacc, mybir
from concourse.bass_utils import run_bass_kernel_spmd

N = 8192
F_IN = 256
H = 64
K = 8
ALPHA = 0.2
N_CORES = 8
R = N // N_CORES          # 1024 rows per core
IB = 2                    # i-blocks per core (512 columns of out-rows each)
IW = R // IB              # 512, i-width per block
G = N // 128              # 64 j-groups of 128
G_SUB = 8                 # j-groups per adjacency DMA
HA = H + 1                # 65: head value columns + denominator column

F32 = mybir.dt.float32
BF16 = mybir.dt.bfloat16
AX = mybir.AluOpType

_cached = {}


def build_program(loop_t=1, z_eng="vvvvvaaa", m_eng="gvgvvvvv",
                  sweeps=((0, 1, 5, 6), (2, 3, 4, 7)), fold_e2=False):
    """z_eng[k]: engine producing head k's score tiles ('v'=DVE zm via
    tensor_scalar mult+max, 'a'=ACT v=relu(G2*g1b-1)=zm-1, 'g'=GPSIMD zm).
    m_eng[k]: engine for the mask multiply ('v' DVE / 'g' GPSIMD).
    ACT heads ('a') use [h*e2|e2] stationary and an extra correction
    matmul with rhs=adj; other heads fold e2 into the score via the
    tensor_scalar scalar2 AP: zm' = max(z*e2, e2).
    sweeps: two groups of 4 heads; each sweep holds 8 PSUM accumulators
    ([65,512] x 2 i-blocks) and re-reads the adjacency."""
    key = (loop_t, z_eng, m_eng, tuple(map(tuple, sweeps)), fold_e2)
    if key in _cached:
        return _cached[key]

    nc = bacc.Bacc("TRN2", target_bir_lowering=False, debug=False,
                   num_devices=N_CORES)

    featT_d = nc.dram_tensor("featT", [F_IN, N], F32, kind="ExternalInput").ap()
    wcat_d = nc.dram_tensor("wcat", [F_IN, K * H], F32, kind="ExternalInput").ap()
    adjr_d = nc.dram_tensor("adjr", [128, G, IB, IW], BF16, kind="ExternalInput").ap()
    g1b_d = nc.dram_tensor("g1b", [128, K, IB, IW], BF16, kind="ExternalInput").ap()
    g2t_d = nc.dram_tensor("g2t", [128, K, G], F32, kind="ExternalInput").ap()
    g2e_d = nc.dram_tensor("g2e", [128, K, G], F32, kind="ExternalInput").ap()
    e25t_d = nc.dram_tensor("e25t", [128, G, K], F32, kind="ExternalInput").ap()
    out_d = nc.dram_tensor("out", [R, K * H], F32, kind="ExternalOutput").ap()

    with tile.TileContext(nc) as tc:
        with ExitStack() as ctx:
            const = ctx.enter_context(tc.tile_pool(name="const", bufs=1))
            hpool = ctx.enter_context(tc.tile_pool(name="h", bufs=G))
            fpool = ctx.enter_context(tc.tile_pool(name="feat", bufs=4))
            e25x_pool = ctx.enter_context(tc.tile_pool(name="e25x", bufs=2))
            adj_pool = ctx.enter_context(tc.tile_pool(name="adj", bufs=2))
            z_pool = ctx.enter_context(tc.tile_pool(name="z", bufs=3))
            u_pool = ctx.enter_context(tc.tile_pool(name="u", bufs=3))
            stg_pool = ctx.enter_context(tc.tile_pool(name="stg", bufs=4))
            fin_pool = ctx.enter_context(tc.tile_pool(name="fin", bufs=2))
            acc_sb_pool = ctx.enter_context(tc.tile_pool(name="accsb", bufs=2))
            psum = ctx.enter_context(tc.tile_pool(name="psum", bufs=8, space="PSUM"))

            # ---- constants ----
            g1b_sb = const.tile([128, K, IB, IW], BF16)
            nc.sync.dma_start(g1b_sb[:], g1b_d[:])
            g2t_sb = const.tile([128, K, G], F32)
            nc.sync.dma_start(g2t_sb[:], g2t_d[:])
            g2e_sb = const.tile([128, K, G], F32)
            nc.sync.dma_start(g2e_sb[:], g2e_d[:])
            e25t_sb = const.tile([128, G, K], F32)
            nc.sync.dma_start(e25t_sb[:], e25t_d[:])
            wcat_sb0 = const.tile([128, K * H], F32)
            nc.sync.dma_start(wcat_sb0[:], wcat_d[0:128, :])
            wcat_sb1 = const.tile([128, K * H], F32)
            nc.sync.dma_start(wcat_sb1[:], wcat_d[128:256, :])
            ident = const.tile([128, 128], F32)
            from concourse.masks import make_identity
            make_identity(nc, ident[:])
            negone = const.tile([128, 1], F32)
            nc.vector.memset(negone[:], -1.0)

            loop_cm = tc.For_i(0, loop_t, 1) if loop_t > 1 else None
            if loop_cm is not None:
                ctx.enter_context(loop_cm)

            # ---- phase A ----
            # ACT-heads ('a'): h_sb slot = [h*e2 | e2];  others: [h | 1]
            # (their e2 is folded into the score tiles via scalar2).
            act_heads = [k for k in range(K) if z_eng[k] == "a" or not fold_e2]
            oth_heads = [k for k in range(K) if z_eng[k] != "a" and fold_e2]
            n_act = len(act_heads)
            h_sb = []
            for g in range(G):
                f0 = fpool.tile([128, 128], F32, tag="f")
                nc.sync.dma_start(f0[:], featT_d[0:128, g * 128:(g + 1) * 128])
                f1 = fpool.tile([128, 128], F32, tag="f")
                nc.sync.dma_start(f1[:], featT_d[128:256, g * 128:(g + 1) * 128])
                ph = psum.tile([128, K * H], F32, tag="ps")
                nc.tensor.matmul(ph[:], f0[:], wcat_sb0[:], start=True, stop=False)
                nc.tensor.matmul(ph[:], f1[:], wcat_sb1[:], start=False, stop=True)

                hg = hpool.tile([128, K, HA], BF16, tag="h")
                phv = ph[:].rearrange("p (k f) -> p k f", k=K)
                n_oth = len(oth_heads)
                a0 = (act_heads[0] if n_act else 0)
                if n_oth:
                    o0 = oth_heads[0]
                    # batched per type (head ranges are contiguous)
                    nc.vector.tensor_copy(hg[:, o0:o0 + n_oth, 0:H],
                                          phv[:, o0:o0 + n_oth, :])
                    nc.gpsimd.memset(hg[:, o0:o0 + n_oth, H:HA], 1.0)
                if n_act:
                    e25x = e25x_pool.tile([128, n_act, H], F32)
                    nc.gpsimd.tensor_copy(
                        e25x[:],
                        e25t_sb[:, g, a0:a0 + n_act]
                        .unsqueeze(2).broadcast_to((128, n_act, H)),
                    )
                    nc.vector.tensor_tensor(
                        hg[:, a0:a0 + n_act, 0:H], phv[:, a0:a0 + n_act, :],
                        e25x[:], op=AX.mult)
                    nc.gpsimd.tensor_copy(hg[:, a0:a0 + n_act, H:HA],
                                          e25t_sb[:, g, a0:a0 + n_act]
                                          .unsqueeze(2))
                h_sb.append(hg)

            # ---- phase B: two 4-head sweeps over the adjacency ----
            assert fold_e2 or len(act_heads) == K
            PAIR = IB * IW  # 1024

            for sweep_heads in sweeps:
                accs = {}
                for k in sweep_heads:
                    for ib in range(IB):
                        accs[(k, ib)] = psum.tile(
                            [HA, IW], F32, tag="ps", name=f"acc{k}_{ib}")
                for gs in range(G // G_SUB):
                    adj_t = adj_pool.tile([128, G_SUB, IB, IW], BF16)
                    nc.sync.dma_start(
                        adj_t[:], adjr_d[:, gs * G_SUB:(gs + 1) * G_SUB, :, :]
                    )
                    for gi in range(G_SUB):
                        g = gs * G_SUB + gi
                        adj_pair = adj_t[:, gi, :, :].rearrange("p b i -> p (b i)")
                        z_all = z_pool.tile([128, 4, PAIR], BF16, tag="z")
                        for s, k in enumerate(sweep_heads):
                            g1b_pair = g1b_sb[:, k, :, :].rearrange(
                                "p b i -> p (b i)")
                            s1 = g2e_sb[:, k, g:g + 1] if fold_e2 else \
                                g2t_sb[:, k, g:g + 1]
                            s2 = e25t_sb[:, g, k:k + 1] if fold_e2 else 1.0
                            if z_eng[k] == "v":
                                # zm' = max(z*e2, e2) -- e2 folded in
                                nc.vector.tensor_scalar(
                                    z_all[:, s, :], g1b_pair, s1, s2,
                                    op0=AX.mult, op1=AX.max,
                                )
                            elif z_eng[k] == "g":
                                nc.gpsimd.tensor_scalar(
                                    z_all[:, s, :], g1b_pair, s1, s2,
                                    op0=AX.mult, op1=AX.max,
                                )
                            else:
                                # v = relu(G2*g1b - 1) = zm - 1 (unscaled)
                                nc.scalar.activation(
                                    z_all[:, s, :], g1b_pair,
                                    mybir.ActivationFunctionType.Relu,
                                    bias=negone[:],
                                    scale=g2t_sb[:, k, g:g + 1],
                                )
                        u_all = u_pool.tile([128, 4, PAIR], BF16, tag="u")
                        for s, k in enumerate(sweep_heads):
                            eng = nc.gpsimd if m_eng[k] == "g" else nc.vector
                            eng.tensor_tensor(
                                u_all[:, s, :], z_all[:, s, :], adj_pair,
                                op=AX.mult)
                        for s, k in enumerate(sweep_heads):
                            is_act = z_eng[k] == "a"
                            for ib in range(IB):
                                nc.tensor.matmul(
                                    accs[(k, ib)][:],
                                    h_sb[g][:, k, :],
                                    u_all[:, s, ib * IW:(ib + 1) * IW],
                                    start=(g == 0),
                                    stop=(g == G - 1 and not is_act),
                                )
                            if is_act:  # correction: acc += h_sb @ adj
                                for ib in range(IB):
                                    nc.tensor.matmul(
                                        accs[(k, ib)][:],
                                        h_sb[g][:, k, :],
                                        adj_t[:, gi, ib, :],
                                        start=False,
                                        stop=(g == G - 1),
                                    )
                # drain this sweep: transpose, divide, elu, store
                sw_sorted = sorted(sweep_heads)
                # contiguous head runs for the output DMA
                runs = []
                for k in sw_sorted:
                    if runs and runs[-1][-1] == k - 1:
                        runs[-1].append(k)
                    else:
                        runs.append([k])
                for ib in range(IB):
                    stgs = [stg_pool.tile([128, 4, HA], F32, tag="stg",
                                          name=f"stg{sweep_heads[0]}_{ib}_{c}")
                            for c in range(IW // 128)]
                    for s, k in enumerate(sw_sorted):
                        acc_sb = acc_sb_pool.tile([HA, IW], F32, tag="accsb")
                        nc.scalar.copy(acc_sb[:], accs[(k, ib)][:])
                        for c in range(IW // 128):
                            pst = psum.tile([128, HA], F32, tag="ps",
                                            name=f"pst{k}_{ib}_{c}")
                            nc.tensor.transpose(
                                pst[:], acc_sb[:, c * 128:(c + 1) * 128],
                                ident[0:HA, 0:HA],
                            )
                            nc.scalar.copy(stgs[c][:, s, :], pst[:])
                    for c in range(IW // 128):
                        stg = stgs[c]
                        recips = fin_pool.tile([128, 4], F32, tag="recip")
                        nc.vector.reciprocal(recips[:], stg[:, :, H])
                        fin = fin_pool.tile([128, 4, H], F32, tag="fin")
                        nc.vector.tensor_tensor(
                            fin[:], stg[:, :, 0:H],
                            recips[:].unsqueeze(2).broadcast_to((128, 4, H)),
                            op=AX.mult,
                        )
                        # elu(x) = exp(min(x,0)) + (max(x,0) - 1)
                        fin2 = fin_pool.tile([128, 4 * H], F32, tag="fin2")
                        finf = fin[:].rearrange("p k f -> p (k f)")
                        nc.vector.tensor_scalar(
                            fin2[:], finf, 0.0, None, op0=AX.min
                        )
                        ex = fin_pool.tile([128, 4 * H], F32, tag="ex")
                        nc.scalar.activation(
                            ex[:], fin2[:], mybir.ActivationFunctionType.Exp
                        )
                        rel = fin_pool.tile([128, 4 * H], F32, tag="rel")
                        nc.vector.tensor_scalar(
                            rel[:], finf, 0.0, -1.0, op0=AX.max, op1=AX.add
                        )
                        res = fin_pool.tile([128, 4 * H], F32, tag="res")
                        nc.vector.tensor_tensor(res[:], ex[:], rel[:], op=AX.add)
                        resv = res[:].rearrange("p (k f) -> p k f", k=4)
                        for run in runs:
                            s0 = sw_sorted.index(run[0])
                            nc.sync.dma_start(
                                out_d[ib * IW + c * 128:
                                      ib * IW + (c + 1) * 128,
                                      run[0] * H:(run[-1] + 1) * H],
                                resv[:, s0:s0 + len(run), :],
                            )

    nc.compile()
    _cached[key] = nc
    return nc


def prepare_inputs(features, adj, W, a):
    """Host-side prep: tiny projections + per-core sharded/transposed layouts."""
    features = np.asarray(features, dtype=np.float32)
    adj = np.asarray(adj, dtype=np.float32)
    W = np.asarray(W, dtype=np.float32)
    a = np.asarray(a, dtype=np.float32)

    # av[k] = W[k] @ a[k]  -> wh = features @ av.T   (tiny: K*F_IN*H flops)
    av1 = np.einsum("kfh,kh->kf", W, a[:, :H])          # [K, F_IN]
    av2 = np.einsum("kfh,kh->kf", W, a[:, H:])          # [K, F_IN]
    wh1 = features @ av1.T                               # [N, K]
    wh2 = features @ av2.T                               # [N, K]
    G1 = np.exp(0.8 * wh1).astype(np.float32)            # row factors
    G2 = np.exp(0.8 * wh2).astype(np.float32)            # col factors
    E25 = np.exp(0.2 * wh2).astype(np.float32)           # folded into values

    featT = np.ascontiguousarray(features.T)             # [F_IN, N]
    wcat = np.ascontiguousarray(
        W.transpose(1, 0, 2).reshape(F_IN, K * H))       # [F_IN, K*H]

    # g2t[p,k,g] = G2[g*128+p, k];  e25t[p,g,k] = E25[g*128+p, k]
    g2t = np.ascontiguousarray(
        G2.reshape(G, 128, K).transpose(1, 2, 0))        # [128, K, G]
    g2e = np.ascontiguousarray(
        (G2 * E25).reshape(G, 128, K).transpose(1, 2, 0))  # [128, K, G]
    e25t = np.ascontiguousarray(
        E25.reshape(G, 128, K).transpose(1, 0, 2))       # [128, G, K]

    in_maps = []
    for c in range(N_CORES):
        r0 = c * R
        # adj_r[p, g, ib, i] = adj[r0 + ib*IW + i, g*128 + p]
        blk = adj[r0:r0 + R, :]                          # [R, N]
        adj_r = np.ascontiguousarray(
            blk.reshape(IB, IW, G, 128).transpose(3, 2, 0, 1)
        ).astype(ml_dtypes.bfloat16)                     # [128, G, IB, IW]
        # g1b[p, k, ib, i] = G1[r0 + ib*IW + i, k]
        g1_blk = G1[r0:r0 + R, :].reshape(IB, IW, K).transpose(2, 0, 1)
        g1b = np.broadcast_to(
            g1_blk[None].astype(ml_dtypes.bfloat16), (128, K, IB, IW))
        g1b = np.ascontiguousarray(g1b)
        in_maps.append({
            "featT": featT,
            "wcat": wcat,
            "adjr": adj_r,
            "g1b": g1b,
            "g2t": g2t,
            "g2e": g2e,
            "e25t": e25t,
        })
    return in_maps


def kernel(features, adj, W, a):
    nc = build_program()
    in_maps = prepare_inputs(features, adj, W, a)
    res = run_bass_kernel_spmd(nc, in_maps, list(range(N_CORES)))
    out = np.concatenate(
        [res.results[c]["out"] for c in range(N_CORES)], axis=0)
    return out.astype(np.float32)


if __name__ == "__main__":
    rng = np.random.default_rng(0)
    features = rng.standard_normal((N, F_IN), dtype=np.float32)
    adj = (rng.integers(0, 2, size=(N, N))).astype(np.float32)
    W = (rng.standard_normal((K, F_IN, H), dtype=np.float32) * 0.118)
    a = (rng.standard_normal((K, 2 * H), dtype=np.float32) * 0.176)
    out = kernel(features=features, adj=adj, W=W, a=a)
    print("out", out.shape, out.dtype, np.abs(out).max())



# revision 8
# speedup vs baseline: 4.4380x; 4.4380x over previous
"""GAT layer (N=8192, F_IN=256, H=64 per head, K=8 heads) on 8 Trainium2 cores.

Strategy (row-sharding, fully data-parallel, no collectives):
  reference per head k:
    h   = features @ W[k]                      [N, H]
    wh1 = h @ a[k,:H]; wh2 = h @ a[k,H:]       [N]
    e   = leaky_relu(wh1[:,None] + wh2[None,:], 0.2)
    att = softmax(where(adj>0, e, -9e15), axis=1)
    out = elu(att @ h)

  Algebra: with s = wh1[i] + wh2[j],
    exp(lrelu(s)) = exp(0.2 s) * max(exp(0.8 s), 1)
                  = e1_{i} * e2_{j} * max(G1_i * G2_j, 1)
  where G1 = exp(0.8 wh1), G2 = exp(0.8 wh2), e1 = exp(0.2 wh1), e2 = exp(0.2 wh2).
  The row factor e1_i cancels in softmax.  The column factor e2_j is folded
  into the value matrix (or the score scalars).  adj in {0,1} multiplies
  exactly:
    unnorm[i,j] = adj[i,j] * max(G1_i G2_j, 1) * e2_j   (up to cancelled e1_i)
    out[i]      = elu( (unnorm @ h) / (unnorm @ 1) )

  Per (head k, col-group g) "unit" the device needs one score op and one
  mask multiply over a [128, 1024] tile, plus a PE matmul pair.  Units are
  scheduled onto engines fractionally per head to balance busy time:
    - score 'v' (DVE tensor_scalar mult+max, 4x mode) or
      score 'a' (ACT relu(G2*g1b - 1) = zm - 1, plus a correction matmul
      acc += h_sb @ adj on the PE)
    - mask multiply on DVE (2x mode) or Pool (GPSIMD)
  Heads that ever use the ACT path carry values [h*e2 | e2]; pure-DVE heads
  carry [h | 1] and fold e2 into the score scalars (s1=G2*e2, s2=e2).

Per-core layout ([j,i]-transposed tiles so contraction j sits on partitions):
  adj_r  [128, 64, 2, 512] bf16 : adj_r[p,g,ib,i] = adj[r0+ib*512+i, g*128+p]
  g1b    [128, 8, 2, 512] bf16  : G1 row broadcast across partitions
  g2t    [128, 8, 64] f32       : g2t[p,k,g] = G2[g*128+p, k]
  g2e    [128, 8, 64] f32       : (G2*E25) likewise (folded-e2 scalars)
  e25t   [128, 64, 8] f32       : e25t[p,g,k] = e2[g*128+p, k]
  featT  [256, 8192] bf16       : features.T (replicated; h computed on device)
  w_cat  [256, 512] bf16        : all heads' W side by side
"""

import sys
import os

sys.path.insert(0, "/opt/trn_rl_repo")

import numpy as np
import ml_dtypes
from contextlib import ExitStack

import concourse.bass as bass
import concourse.tile as tile
from concourse import bacc, mybir
from concourse.bass_utils import run_bass_kernel_spmd

N = 8192
F_IN = 256
H = 64
K = 8
ALPHA = 0.2
N_CORES = 8
R = N // N_CORES          # 1024 rows per core
IB = 2                    # i-blocks per core (512 columns of out-rows each)
IW = R // IB              # 512, i-width per block
G = N // 128              # 64 j-groups of 128
G_SUB = 8                 # j-groups per adjacency DMA
HA = H + 1                # 65: head value columns + denominator column

F32 = mybir.dt.float32
BF16 = mybir.dt.bfloat16
AX = mybir.AluOpType

_cached = {}

# fraction of each head's 64 column-groups whose score runs on ACT
Z_ACT_DEFAULT = (0.0, 0.0, 0.0, 0.0, 0.5, 1.0, 1.0, 1.0)
# fraction of each head's groups whose mask multiply runs on Pool/GPSIMD
M_POOL_DEFAULT = (1.0, 0.125, 1.0, 0.0, 0.0, 0.0, 0.0, 0.0)


def _spread(frac, n):
    """Evenly-spread boolean schedule of round(frac*n) True slots."""
    return [int((g + 1) * frac) - int(g * frac) > 0 for g in range(n)]


def build_program(loop_t=1, z_act=Z_ACT_DEFAULT, m_pool=M_POOL_DEFAULT,
                  sweeps=((0, 1, 5, 6), (2, 3, 4, 7))):
    key = (loop_t, tuple(z_act), tuple(m_pool), tuple(map(tuple, sweeps)))
    if key in _cached:
        return _cached[key]

    z_act_gs = [_spread(z_act[k], G) for k in range(K)]
    m_pool_gs = [_spread(m_pool[k], G) for k in range(K)]
    # heads that ever use the ACT score path carry [h*e2 | e2] values
    acty = [any(z_act_gs[k]) for k in range(K)]
    act_heads = [k for k in range(K) if acty[k]]
    oth_heads = [k for k in range(K) if not acty[k]]
    # slot layouts are built per contiguous range
    assert act_heads == list(range(min(act_heads), max(act_heads) + 1)) if \
        act_heads else True
    assert oth_heads == list(range(min(oth_heads), max(oth_heads) + 1)) if \
        oth_heads else True

    nc = bacc.Bacc("TRN2", target_bir_lowering=False, debug=False,
                   num_devices=N_CORES)

    featT_d = nc.dram_tensor("featT", [F_IN, N], BF16, kind="ExternalInput").ap()
    wcat_d = nc.dram_tensor("wcat", [F_IN, K * H], BF16, kind="ExternalInput").ap()
    adjr_d = nc.dram_tensor("adjr", [128, G, IB, IW], BF16, kind="ExternalInput").ap()
    g1b_d = nc.dram_tensor("g1b", [128, K, IB, IW], BF16, kind="ExternalInput").ap()
    g2t_d = nc.dram_tensor("g2t", [128, K, G], F32, kind="ExternalInput").ap()
    g2e_d = nc.dram_tensor("g2e", [128, K, G], F32, kind="ExternalInput").ap()
    e25t_d = nc.dram_tensor("e25t", [128, G, K], F32, kind="ExternalInput").ap()
    out_d = nc.dram_tensor("out", [R, K * H], F32, kind="ExternalOutput").ap()

    with tile.TileContext(nc) as tc:
        with ExitStack() as ctx:
            const = ctx.enter_context(tc.tile_pool(name="const", bufs=1))
            hpool = ctx.enter_context(tc.tile_pool(name="h", bufs=G))
            fpool = ctx.enter_context(tc.tile_pool(name="feat", bufs=4))
            e25x_pool = ctx.enter_context(tc.tile_pool(name="e25x", bufs=2))
            adj_pool = ctx.enter_context(tc.tile_pool(name="adj", bufs=2))
            z_pool = ctx.enter_context(tc.tile_pool(name="z", bufs=3))
            u_pool = ctx.enter_context(tc.tile_pool(name="u", bufs=3))
            stg_pool = ctx.enter_context(tc.tile_pool(name="stg", bufs=4))
            fin_pool = ctx.enter_context(tc.tile_pool(name="fin", bufs=2))
            acc_sb_pool = ctx.enter_context(tc.tile_pool(name="accsb", bufs=2))
            psum = ctx.enter_context(tc.tile_pool(name="psum", bufs=8, space="PSUM"))

            # ---- constants ----
            g1b_sb = const.tile([128, K, IB, IW], BF16)
            nc.sync.dma_start(g1b_sb[:], g1b_d[:])
            g2t_sb = const.tile([128, K, G], F32)
            nc.sync.dma_start(g2t_sb[:], g2t_d[:])
            g2e_sb = const.tile([128, K, G], F32)
            nc.sync.dma_start(g2e_sb[:], g2e_d[:])
            e25t_sb = const.tile([128, G, K], F32)
            nc.sync.dma_start(e25t_sb[:], e25t_d[:])
            wcat_sb0 = const.tile([128, K * H], BF16)
            nc.sync.dma_start(wcat_sb0[:], wcat_d[0:128, :])
            wcat_sb1 = const.tile([128, K * H], BF16)
            nc.sync.dma_start(wcat_sb1[:], wcat_d[128:256, :])
            ident = const.tile([128, 128], F32)
            from concourse.masks import make_identity
            make_identity(nc, ident[:])
            negone = const.tile([128, 1], F32)
            nc.vector.memset(negone[:], -1.0)

            loop_cm = tc.For_i(0, loop_t, 1) if loop_t > 1 else None
            if loop_cm is not None:
                ctx.enter_context(loop_cm)

            # ---- phase A ----
            # act-ish heads: h_sb slot = [h*e2 | e2]; pure-v: [h | 1]
            n_act = len(act_heads)
            GF = 8                       # j-groups per featT DMA slab
            h_sb = []
            fslabs = {}
            for g in range(G):
                if g % GF == 0:
                    f0 = fpool.tile([128, GF * 128], BF16, tag="f")
                    nc.sync.dma_start(
                        f0[:], featT_d[0:128, g * 128:(g + GF) * 128])
                    f1 = fpool.tile([128, GF * 128], BF16, tag="f")
                    nc.sync.dma_start(
                        f1[:], featT_d[128:256, g * 128:(g + GF) * 128])
                    fslabs = (f0, f1)
                go = (g % GF) * 128
                ph = psum.tile([128, K * H], F32, tag="ps")
                nc.tensor.matmul(ph[:], fslabs[0][:, go:go + 128],
                                 wcat_sb0[:], start=True, stop=False)
                nc.tensor.matmul(ph[:], fslabs[1][:, go:go + 128],
                                 wcat_sb1[:], start=False, stop=True)

                hg = hpool.tile([128, K, HA], BF16, tag="h")
                phv = ph[:].rearrange("p (k f) -> p k f", k=K)
                n_oth = len(oth_heads)
                a0 = (act_heads[0] if n_act else 0)
                if n_oth:
                    o0 = oth_heads[0]
                    nc.scalar.copy(hg[:, o0:o0 + n_oth, 0:H],
                                   phv[:, o0:o0 + n_oth, :])
                    nc.gpsimd.memset(hg[:, o0:o0 + n_oth, H:HA], 1.0)
                if n_act:
                    nc.gpsimd.tensor_tensor(
                        hg[:, a0:a0 + n_act, 0:H], phv[:, a0:a0 + n_act, :],
                        e25t_sb[:, g, a0:a0 + n_act]
                        .unsqueeze(2).broadcast_to((128, n_act, H)),
                        op=AX.mult)
                    nc.gpsimd.tensor_copy(hg[:, a0:a0 + n_act, H:HA],
                                          e25t_sb[:, g, a0:a0 + n_act]
                                          .unsqueeze(2))
                h_sb.append(hg)

            # ---- phase B: two 4-head sweeps over the adjacency ----
            PAIR = IB * IW  # 1024

            for sweep_heads in sweeps:
                accs = {}
                for k in sweep_heads:
                    for ib in range(IB):
                        accs[(k, ib)] = psum.tile(
                            [HA, IW], F32, tag="ps", name=f"acc{k}_{ib}")
                for gs in range(G // G_SUB):
                    adj_t = adj_pool.tile([128, G_SUB, IB, IW], BF16)
                    nc.sync.dma_start(
                        adj_t[:], adjr_d[:, gs * G_SUB:(gs + 1) * G_SUB, :, :]
                    )
                    for gi in range(G_SUB):
                        g = gs * G_SUB + gi
                        adj_pair = adj_t[:, gi, :, :].rearrange("p b i -> p (b i)")
                        z_all = z_pool.tile([128, 4, PAIR], BF16, tag="z")
                        for s, k in enumerate(sweep_heads):
                            g1b_pair = g1b_sb[:, k, :, :].rearrange(
                                "p b i -> p (b i)")
                            if z_act_gs[k][g]:
                                # z' = relu(G2*g1b - 1) = zm - 1 (+corr matmul)
                                nc.scalar.activation(
                                    z_all[:, s, :], g1b_pair,
                                    mybir.ActivationFunctionType.Relu,
                                    bias=negone[:],
                                    scale=g2t_sb[:, k, g:g + 1],
                                )
                            elif acty[k]:
                                # values carry e2 -> plain zm = max(z*G2, 1)
                                nc.vector.tensor_scalar(
                                    z_all[:, s, :], g1b_pair,
                                    g2t_sb[:, k, g:g + 1], 1.0,
                                    op0=AX.mult, op1=AX.max,
                                )
                            else:
                                # e2 folded into score: zm' = max(z*G2e2, e2)
                                nc.vector.tensor_scalar(
                                    z_all[:, s, :], g1b_pair,
                                    g2e_sb[:, k, g:g + 1],
                                    e25t_sb[:, g, k:k + 1],
                                    op0=AX.mult, op1=AX.max,
                                )
                        u_all = u_pool.tile([128, 4, PAIR], BF16, tag="u")
                        for s, k in enumerate(sweep_heads):
                            eng = nc.gpsimd if m_pool_gs[k][g] else nc.vector
                            eng.tensor_tensor(
                                u_all[:, s, :], z_all[:, s, :], adj_pair,
                                op=AX.mult)
                        for s, k in enumerate(sweep_heads):
                            is_act = z_act_gs[k][g]
                            # ensure the g=G-1 chain ends on the right instr
                            for ib in range(IB):
                                nc.tensor.matmul(
                                    accs[(k, ib)][:],
                                    h_sb[g][:, k, :],
                                    u_all[:, s, ib * IW:(ib + 1) * IW],
                                    start=(g == 0),
                                    stop=(g == G - 1 and not is_act),
                                )
                            if is_act:  # correction: acc += h_sb @ adj
                                for ib in range(IB):
                                    nc.tensor.matmul(
                                        accs[(k, ib)][:],
                                        h_sb[g][:, k, :],
                                        adj_t[:, gi, ib, :],
                                        start=False,
                                        stop=(g == G - 1),
                                    )
                # drain this sweep: transpose, divide, elu, store
                sw_sorted = sorted(sweep_heads)
                # contiguous head runs for the output DMA
                runs = []
                for k in sw_sorted:
                    if runs and runs[-1][-1] == k - 1:
                        runs[-1].append(k)
                    else:
                        runs.append([k])
                for ib in range(IB):
                    acc_sbs = {}
                    for s, k in enumerate(sw_sorted):
                        acc_sb = acc_sb_pool.tile([HA, IW], F32, tag="accsb")
                        nc.sync.dma_start(acc_sb[:], accs[(k, ib)][:])
                        acc_sbs[k] = acc_sb
                    stgs = []
                    for c in range(IW // 128):
                        pst = psum.tile([128, 4, HA], F32, tag="ps",
                                        name=f"pst{sweep_heads[0]}_{ib}_{c}")
                        for s, k in enumerate(sw_sorted):
                            nc.tensor.transpose(
                                pst[:, s, :],
                                acc_sbs[k][:, c * 128:(c + 1) * 128],
                                ident[0:HA, 0:HA],
                            )
                        stg = stg_pool.tile([128, 4, HA], F32, tag="stg",
                                            name=f"stg{sweep_heads[0]}_{ib}_{c}")
                        nc.scalar.copy(stg[:], pst[:])
                        stgs.append(stg)
                    for c in range(IW // 128):
                        stg = stgs[c]
                        recips = fin_pool.tile([128, 4], F32, tag="recip")
                        nc.vector.reciprocal(recips[:], stg[:, :, H])
                        fin = fin_pool.tile([128, 4, H], F32, tag="fin")
                        nc.vector.tensor_tensor(
                            fin[:], stg[:, :, 0:H],
                            recips[:].unsqueeze(2).broadcast_to((128, 4, H)),
                            op=AX.mult,
                        )
                        # elu(x) = exp(min(x,0)) + (max(x,0) - 1)
                        fin2 = fin_pool.tile([128, 4 * H], F32, tag="fin2")
                        finf = fin[:].rearrange("p k f -> p (k f)")
                        nc.vector.tensor_scalar(
                            fin2[:], finf, 0.0, None, op0=AX.min
                        )
                        ex = fin_pool.tile([128, 4 * H], F32, tag="ex")
                        nc.scalar.activation(
                            ex[:], fin2[:], mybir.ActivationFunctionType.Exp
                        )
                        rel = fin_pool.tile([128, 4 * H], F32, tag="rel")
                        nc.vector.tensor_scalar(
                            rel[:], finf, 0.0, -1.0, op0=AX.max, op1=AX.add
                        )
                        res = fin_pool.tile([128, 4 * H], F32, tag="res")
                        nc.vector.tensor_tensor(res[:], ex[:], rel[:], op=AX.add)
                        resv = res[:].rearrange("p (k f) -> p k f", k=4)
                        for run in runs:
                            s0 = sw_sorted.index(run[0])
                            nc.sync.dma_start(
                                out_d[ib * IW + c * 128:
                                      ib * IW + (c + 1) * 128,
                                      run[0] * H:(run[-1] + 1) * H],
                                resv[:, s0:s0 + len(run), :],
                            )

    nc.compile()
    _cached[key] = nc
    return nc


def prepare_inputs(features, adj, W, a):
    """Host-side prep: tiny projections + per-core sharded/transposed layouts."""
    features = np.asarray(features, dtype=np.float32)
    adj = np.asarray(adj, dtype=np.float32)
    W = np.asarray(W, dtype=np.float32)
    a = np.asarray(a, dtype=np.float32)

    # av[k] = W[k] @ a[k]  -> wh = features @ av.T   (tiny: K*F_IN*H flops)
    av1 = np.einsum("kfh,kh->kf", W, a[:, :H])          # [K, F_IN]
    av2 = np.einsum("kfh,kh->kf", W, a[:, H:])          # [K, F_IN]
    wh1 = features @ av1.T                               # [N, K]
    wh2 = features @ av2.T                               # [N, K]
    G1 = np.exp(0.8 * wh1).astype(np.float32)            # row factors
    G2 = np.exp(0.8 * wh2).astype(np.float32)            # col factors
    E25 = np.exp(0.2 * wh2).astype(np.float32)           # folded into values

    featT = np.ascontiguousarray(features.T).astype(ml_dtypes.bfloat16)
    wcat = np.ascontiguousarray(
        W.transpose(1, 0, 2).reshape(F_IN, K * H)).astype(ml_dtypes.bfloat16)

    # g2t[p,k,g] = G2[g*128+p, k];  e25t[p,g,k] = E25[g*128+p, k]
    g2t = np.ascontiguousarray(
        G2.reshape(G, 128, K).transpose(1, 2, 0))        # [128, K, G]
    g2e = np.ascontiguousarray(
        (G2 * E25).reshape(G, 128, K).transpose(1, 2, 0))  # [128, K, G]
    e25t = np.ascontiguousarray(
        E25.reshape(G, 128, K).transpose(1, 0, 2))       # [128, G, K]

    in_maps = []
    for c in range(N_CORES):
        r0 = c * R
        # adj_r[p, g, ib, i] = adj[r0 + ib*IW + i, g*128 + p]
        blk = adj[r0:r0 + R, :]                          # [R, N]
        adj_r = np.ascontiguousarray(
            blk.reshape(IB, IW, G, 128).transpose(3, 2, 0, 1)
        ).astype(ml_dtypes.bfloat16)                     # [128, G, IB, IW]
        # g1b[p, k, ib, i] = G1[r0 + ib*IW + i, k]
        g1_blk = G1[r0:r0 + R, :].reshape(IB, IW, K).transpose(2, 0, 1)
        g1b = np.broadcast_to(
            g1_blk[None].astype(ml_dtypes.bfloat16), (128, K, IB, IW))
        g1b = np.ascontiguousarray(g1b)
        in_maps.append({
            "featT": featT,
            "wcat": wcat,
            "adjr": adj_r,
            "g1b": g1b,
            "g2t": g2t,
            "g2e": g2e,
            "e25t": e25t,
        })
    return in_maps


def kernel(features, adj, W, a):
    nc = build_program()
    in_maps = prepare_inputs(features, adj, W, a)
    res = run_bass_kernel_spmd(nc, in_maps, list(range(N_CORES)))
    out = np.concatenate(
        [res.results[c]["out"] for c in range(N_CORES)], axis=0)
    return out.astype(np.float32)


if __name__ == "__main__":
    rng = np.random.default_rng(0)
    features = rng.standard_normal((N, F_IN), dtype=np.float32)
    adj = (rng.integers(0, 2, size=(N, N))).astype(np.float32)
    W = (rng.standard_normal((K, F_IN, H), dtype=np.float32) * 0.118)
    a = (rng.standard_normal((K, 2 * H), dtype=np.float32) * 0.176)
    out = kernel(features=features, adj=adj, W=W, a=a)
    print("out", out.shape, out.dtype, np.abs(out).max())


# revision 24
# speedup vs baseline: 4.7254x; 1.0647x over previous
"""GAT layer (N=8192, F_IN=256, H=64 per head, K=8 heads) on 8 Trainium2 cores.

Strategy (row-sharding, fully data-parallel, no collectives):
  reference per head k:
    h   = features @ W[k]                      [N, H]
    wh1 = h @ a[k,:H]; wh2 = h @ a[k,H:]       [N]
    e   = leaky_relu(wh1[:,None] + wh2[None,:], 0.2)
    att = softmax(where(adj>0, e, -9e15), axis=1)
    out = elu(att @ h)

  Algebra: with s = wh1[i] + wh2[j],
    exp(lrelu(s)) = exp(0.2 s) * max(exp(0.8 s), 1)
                  = e1_{i} * e2_{j} * max(G1_i * G2_j, 1)
  where G1 = exp(0.8 wh1), G2 = exp(0.8 wh2), e1 = exp(0.2 wh1), e2 = exp(0.2 wh2).
  The row factor e1_i cancels in softmax.  The column factor e2_j is folded
  into the value matrix (or the score scalars).  adj in {0,1} multiplies
  exactly:
    unnorm[i,j] = adj[i,j] * max(G1_i G2_j, 1) * e2_j   (up to cancelled e1_i)
    out[i]      = elu( (unnorm @ h) / (unnorm @ 1) )

  Per (head k, col-group g) "unit" the device needs one score op and one
  mask multiply over a [128, 1024] tile, plus a PE matmul pair.  Units are
  scheduled onto engines fractionally per head to balance busy time:
    - score 'v' (DVE tensor_scalar mult+max, 4x mode) or
      score 'a' (ACT relu(G2*g1b - 1) = zm - 1, plus a correction matmul
      acc += h_sb @ adj on the PE)
    - mask multiply on DVE (2x mode) or Pool (GPSIMD)
  Heads that ever use the ACT path carry values [h*e2 | e2]; pure-DVE heads
  carry [h | 1] and fold e2 into the score scalars (s1=G2*e2, s2=e2).

Per-core layout ([j,i]-transposed tiles so contraction j sits on partitions):
  adj_r  [128, 64, 2, 512] bf16 : adj_r[p,g,ib,i] = adj[r0+ib*512+i, g*128+p]
  g1b    [128, 8, 2, 512] bf16  : G1 row broadcast across partitions
  g2t    [128, 8, 64] f32       : g2t[p,k,g] = G2[g*128+p, k]
  g2e    [128, 8, 64] f32       : (G2*E25) likewise (folded-e2 scalars)
  e25t   [128, 64, 8] f32       : e25t[p,g,k] = e2[g*128+p, k]
  featT  [256, 8192] bf16       : features.T (replicated; h computed on device)
  w_cat  [256, 512] bf16        : all heads' W side by side
"""

import sys
import os

sys.path.insert(0, "/opt/trn_rl_repo")

import numpy as np
import ml_dtypes
from contextlib import ExitStack

import concourse.bass as bass
import concourse.tile as tile
from concourse import bacc, mybir
from concourse.bass_utils import run_bass_kernel_spmd

N = 8192
F_IN = 256
H = 64
K = 8
ALPHA = 0.2
N_CORES = 8
R = N // N_CORES          # 1024 rows per core
IB = 2                    # i-blocks per core (512 columns of out-rows each)
IW = R // IB              # 512, i-width per block
G = N // 128              # 64 j-groups of 128
G_SUB = 8                 # j-groups per adjacency DMA
HA = H + 1                # 65: head value columns + denominator column

F32 = mybir.dt.float32
BF16 = mybir.dt.bfloat16
AX = mybir.AluOpType

_cached = {}

# fraction of each head's 64 column-groups whose score runs on ACT
Z_ACT_DEFAULT = (0.0, 0.0, 0.0, 0.0, 1.0, 1.0, 1.0, 1.0)
# fraction of each head's groups whose mask multiply runs on Pool/GPSIMD
M_POOL_DEFAULT = (1.0, 0.125, 1.0, 0.0, 0.0, 0.0, 0.0, 0.0)


def _spread(frac, n):
    """Evenly-spread boolean schedule of round(frac*n) True slots."""
    return [int((g + 1) * frac) - int(g * frac) > 0 for g in range(n)]


def build_program(loop_t=1, z_act=Z_ACT_DEFAULT, m_pool=M_POOL_DEFAULT,
                  sweeps=((0, 1, 5, 6), (2, 3, 4, 7)), pa_mode="dve",
                  drain_mode="act", lag=4):
    key = (loop_t, tuple(z_act), tuple(m_pool), tuple(map(tuple, sweeps)),
           pa_mode, drain_mode, lag)
    if key in _cached:
        return _cached[key]

    z_act_gs = [_spread(z_act[k], G) for k in range(K)]
    m_pool_gs = [_spread(m_pool[k], G) for k in range(K)]
    # heads that ever use the ACT score path carry [h*e2 | e2] values
    acty = [any(z_act_gs[k]) for k in range(K)]
    act_heads = [k for k in range(K) if acty[k]]
    oth_heads = [k for k in range(K) if not acty[k]]
    # slot layouts are built per contiguous range
    assert act_heads == list(range(min(act_heads), max(act_heads) + 1)) if \
        act_heads else True
    assert oth_heads == list(range(min(oth_heads), max(oth_heads) + 1)) if \
        oth_heads else True

    nc = bacc.Bacc("TRN2", target_bir_lowering=False, debug=False,
                   num_devices=N_CORES)

    featT_d = nc.dram_tensor("featT", [F_IN, N], BF16, kind="ExternalInput").ap()
    wcat_d = nc.dram_tensor("wcat", [F_IN, K * H], BF16, kind="ExternalInput").ap()
    adjr_d = nc.dram_tensor("adjr", [128, G, IB, IW], BF16, kind="ExternalInput").ap()
    g1b_d = nc.dram_tensor("g1b", [128, K, IB, IW], BF16, kind="ExternalInput").ap()
    g2t_d = nc.dram_tensor("g2t", [128, K, G], F32, kind="ExternalInput").ap()
    g2e_d = nc.dram_tensor("g2e", [128, K, G], F32, kind="ExternalInput").ap()
    e25t_d = nc.dram_tensor("e25t", [128, G, K], F32, kind="ExternalInput").ap()
    out_d = nc.dram_tensor("out", [R, K * H], F32, kind="ExternalOutput").ap()

    with tile.TileContext(nc) as tc:
        with ExitStack() as ctx:
            const = ctx.enter_context(tc.tile_pool(name="const", bufs=1))
            hpool = ctx.enter_context(tc.tile_pool(name="h", bufs=G))
            fpool = ctx.enter_context(tc.tile_pool(name="feat", bufs=4))
            e25x_pool = ctx.enter_context(tc.tile_pool(name="e25x", bufs=2))
            adj_pool = ctx.enter_context(tc.tile_pool(name="adj", bufs=2))
            z_pool = ctx.enter_context(tc.tile_pool(name="z", bufs=3))
            u_pool = ctx.enter_context(tc.tile_pool(name="u", bufs=3))
            stg_pool = ctx.enter_context(tc.tile_pool(name="stg", bufs=4))
            fin_pool = ctx.enter_context(tc.tile_pool(name="fin", bufs=2))
            acc_sb_pool = ctx.enter_context(tc.tile_pool(name="accsb", bufs=2))
            psum = ctx.enter_context(tc.tile_pool(name="psum", bufs=8, space="PSUM"))

            # ---- constants ----
            g1b_sb = const.tile([128, K, IB, IW], BF16)
            nc.sync.dma_start(g1b_sb[:], g1b_d[:])
            g2t_sb = const.tile([128, K, G], F32)
            nc.sync.dma_start(g2t_sb[:], g2t_d[:])
            g2e_sb = const.tile([128, K, G], F32)
            nc.sync.dma_start(g2e_sb[:], g2e_d[:])
            e25t_sb = const.tile([128, G, K], F32)
            nc.sync.dma_start(e25t_sb[:], e25t_d[:])
            wcat_sb0 = const.tile([128, K * H], BF16)
            nc.sync.dma_start(wcat_sb0[:], wcat_d[0:128, :])
            wcat_sb1 = const.tile([128, K * H], BF16)
            nc.sync.dma_start(wcat_sb1[:], wcat_d[128:256, :])
            ident = const.tile([128, 128], F32)
            from concourse.masks import make_identity
            make_identity(nc, ident[:])
            negone = const.tile([128, 1], F32)
            nc.vector.memset(negone[:], -1.0)

            loop_cm = tc.For_i(0, loop_t, 1) if loop_t > 1 else None
            if loop_cm is not None:
                ctx.enter_context(loop_cm)

            # ---- phase A ----
            # act-ish heads: h_sb slot = [h*e2 | e2]; pure-v: [h | 1]
            n_act = len(act_heads)
            GF = 8                       # j-groups per featT DMA slab
            h_sb = []
            fslabs = {}
            for g in range(G):
                if g % GF == 0:
                    f0 = fpool.tile([128, GF * 128], BF16, tag="f")
                    nc.sync.dma_start(
                        f0[:], featT_d[0:128, g * 128:(g + GF) * 128])
                    f1 = fpool.tile([128, GF * 128], BF16, tag="f")
                    nc.sync.dma_start(
                        f1[:], featT_d[128:256, g * 128:(g + GF) * 128])
                    fslabs = (f0, f1)
                go = (g % GF) * 128
                ph = psum.tile([128, K * H], F32, tag="ps")
                nc.tensor.matmul(ph[:], fslabs[0][:, go:go + 128],
                                 wcat_sb0[:], start=True, stop=False)
                nc.tensor.matmul(ph[:], fslabs[1][:, go:go + 128],
                                 wcat_sb1[:], start=False, stop=True)

                hg = hpool.tile([128, K, HA], BF16, tag="h")
                phv = ph[:].rearrange("p (k f) -> p k f", k=K)
                n_oth = len(oth_heads)
                a0 = (act_heads[0] if n_act else 0)
                if n_oth:
                    o0 = oth_heads[0]
                    nc.scalar.copy(hg[:, o0:o0 + n_oth, 0:H],
                                   phv[:, o0:o0 + n_oth, :])
                    nc.gpsimd.memset(hg[:, o0:o0 + n_oth, H:HA], 1.0)
                if n_act:
                    e25b = (e25t_sb[:, g, a0:a0 + n_act]
                            .unsqueeze(2).broadcast_to((128, n_act, H)))
                    if pa_mode == "pool":
                        nc.gpsimd.tensor_tensor(
                            hg[:, a0:a0 + n_act, 0:H],
                            phv[:, a0:a0 + n_act, :], e25b, op=AX.mult)
                    else:
                        nc.vector.tensor_tensor(
                            hg[:, a0:a0 + n_act, 0:H],
                            phv[:, a0:a0 + n_act, :], e25b, op=AX.mult)
                    nc.gpsimd.tensor_copy(hg[:, a0:a0 + n_act, H:HA],
                                          e25t_sb[:, g, a0:a0 + n_act]
                                          .unsqueeze(2))
                h_sb.append(hg)

            # ---- phase B: two 4-head sweeps over the adjacency ----
            PAIR = IB * IW  # 1024

            for sweep_heads in sweeps:
                accs = {}
                for k in sweep_heads:
                    for ib in range(IB):
                        accs[(k, ib)] = psum.tile(
                            [HA, IW], F32, tag="ps", name=f"acc{k}_{ib}")
                for gs in range(G // G_SUB):
                    adj_t = adj_pool.tile([128, G_SUB, IB, IW], BF16)
                    nc.sync.dma_start(
                        adj_t[:], adjr_d[:, gs * G_SUB:(gs + 1) * G_SUB, :, :]
                    )
                    for gi in range(G_SUB):
                        g = gs * G_SUB + gi
                        adj_pair = adj_t[:, gi, :, :].rearrange("p b i -> p (b i)")
                        z_all = z_pool.tile([128, 4, PAIR], BF16, tag="z")
                        for s, k in enumerate(sweep_heads):
                            g1b_pair = g1b_sb[:, k, :, :].rearrange(
                                "p b i -> p (b i)")
                            if z_act_gs[k][g]:
                                # z' = relu(G2*g1b - 1) = zm - 1 (+corr matmul)
                                nc.scalar.activation(
                                    z_all[:, s, :], g1b_pair,
                                    mybir.ActivationFunctionType.Relu,
                                    bias=negone[:],
                                    scale=g2t_sb[:, k, g:g + 1],
                                )
                            elif acty[k]:
                                # values carry e2 -> plain zm = max(z*G2, 1)
                                nc.vector.tensor_scalar(
                                    z_all[:, s, :], g1b_pair,
                                    g2t_sb[:, k, g:g + 1], 1.0,
                                    op0=AX.mult, op1=AX.max,
                                )
                            else:
                                # e2 folded into score: zm' = max(z*G2e2, e2)
                                nc.vector.tensor_scalar(
                                    z_all[:, s, :], g1b_pair,
                                    g2e_sb[:, k, g:g + 1],
                                    e25t_sb[:, g, k:k + 1],
                                    op0=AX.mult, op1=AX.max,
                                )
                        u_all = u_pool.tile([128, 4, PAIR], BF16, tag="u")
                        for s, k in enumerate(sweep_heads):
                            eng = nc.gpsimd if m_pool_gs[k][g] else nc.vector
                            eng.tensor_tensor(
                                u_all[:, s, :], z_all[:, s, :], adj_pair,
                                op=AX.mult)
                        for s, k in enumerate(sweep_heads):
                            is_act = z_act_gs[k][g]
                            for ib in range(IB):
                                nc.tensor.matmul(
                                    accs[(k, ib)][:],
                                    h_sb[g][:, k, :],
                                    u_all[:, s, ib * IW:(ib + 1) * IW],
                                    start=(g == 0),
                                    stop=(g == G - 1 and not is_act),
                                )
                            if is_act:  # correction: acc += h_sb @ adj
                                for ib in range(IB):
                                    nc.tensor.matmul(
                                        accs[(k, ib)][:],
                                        h_sb[g][:, k, :],
                                        adj_t[:, gi, ib, :],
                                        start=False,
                                        stop=(g == G - 1),
                                    )
                # drain this sweep: transpose, divide, elu, store
                sw_sorted = sorted(sweep_heads)
                runs = []
                for k in sw_sorted:
                    if runs and runs[-1][-1] == k - 1:
                        runs[-1].append(k)
                    else:
                        runs.append([k])
                for ib in range(IB):
                    stgs = [stg_pool.tile([128, 4, HA], F32, tag="stg",
                                          name=f"stg{sweep_heads[0]}_{ib}_{c}")
                            for c in range(IW // 128)]
                    for s, k in enumerate(sw_sorted):
                        acc_sb = acc_sb_pool.tile([HA, IW], F32, tag="accsb")
                        nc.scalar.copy(acc_sb[:], accs[(k, ib)][:])
                        for c in range(IW // 128):
                            pst = psum.tile([128, HA], F32, tag="ps",
                                            name=f"pst{k}_{ib}_{c}")
                            nc.tensor.transpose(
                                pst[:], acc_sb[:, c * 128:(c + 1) * 128],
                                ident[0:HA, 0:HA],
                            )
                            nc.scalar.copy(stgs[c][:, s, :], pst[:])
                    for c in range(IW // 128):
                        stg = stgs[c]
                        recips = fin_pool.tile([128, 4], F32, tag="recip")
                        nc.vector.reciprocal(recips[:], stg[:, :, H])
                        fin = fin_pool.tile([128, 4, H], F32, tag="fin")
                        nc.vector.tensor_tensor(
                            fin[:], stg[:, :, 0:H],
                            recips[:].unsqueeze(2).broadcast_to((128, 4, H)),
                            op=AX.mult,
                        )
                        # elu(x) = exp(min(x,0)) + (max(x,0) - 1)
                        fin2 = fin_pool.tile([128, 4 * H], F32, tag="fin2")
                        finf = fin[:].rearrange("p k f -> p (k f)")
                        nc.vector.tensor_scalar(
                            fin2[:], finf, 0.0, None, op0=AX.min
                        )
                        ex = fin_pool.tile([128, 4 * H], F32, tag="ex")
                        nc.scalar.activation(
                            ex[:], fin2[:], mybir.ActivationFunctionType.Exp
                        )
                        rel = fin_pool.tile([128, 4 * H], F32, tag="rel")
                        nc.vector.tensor_scalar(
                            rel[:], finf, 0.0, -1.0, op0=AX.max, op1=AX.add
                        )
                        res = fin_pool.tile([128, 4 * H], F32, tag="res")
                        nc.vector.tensor_tensor(res[:], ex[:], rel[:], op=AX.add)
                        resv = res[:].rearrange("p (k f) -> p k f", k=4)
                        for run in runs:
                            s0 = sw_sorted.index(run[0])
                            nc.sync.dma_start(
                                out_d[ib * IW + c * 128:
                                      ib * IW + (c + 1) * 128,
                                      run[0] * H:(run[-1] + 1) * H],
                                resv[:, s0:s0 + len(run), :],
                            )

    nc.compile()
    _cached[key] = nc
    return nc


def prepare_inputs(features, adj, W, a):
    """Host-side prep: tiny projections + per-core sharded/transposed layouts."""
    features = np.asarray(features, dtype=np.float32)
    adj = np.asarray(adj, dtype=np.float32)
    W = np.asarray(W, dtype=np.float32)
    a = np.asarray(a, dtype=np.float32)

    # av[k] = W[k] @ a[k]  -> wh = features @ av.T   (tiny: K*F_IN*H flops)
    av1 = np.einsum("kfh,kh->kf", W, a[:, :H])          # [K, F_IN]
    av2 = np.einsum("kfh,kh->kf", W, a[:, H:])          # [K, F_IN]
    wh1 = features @ av1.T                               # [N, K]
    wh2 = features @ av2.T                               # [N, K]
    G1 = np.exp(0.8 * wh1).astype(np.float32)            # row factors
    G2 = np.exp(0.8 * wh2).astype(np.float32)            # col factors
    E25 = np.exp(0.2 * wh2).astype(np.float32)           # folded into values

    featT = np.ascontiguousarray(features.T).astype(ml_dtypes.bfloat16)
    wcat = np.ascontiguousarray(
        W.transpose(1, 0, 2).reshape(F_IN, K * H)).astype(ml_dtypes.bfloat16)

    # g2t[p,k,g] = G2[g*128+p, k];  e25t[p,g,k] = E25[g*128+p, k]
    g2t = np.ascontiguousarray(
        G2.reshape(G, 128, K).transpose(1, 2, 0))        # [128, K, G]
    g2e = np.ascontiguousarray(
        (G2 * E25).reshape(G, 128, K).transpose(1, 2, 0))  # [128, K, G]
    e25t = np.ascontiguousarray(
        E25.reshape(G, 128, K).transpose(1, 0, 2))       # [128, G, K]

    in_maps = []
    for c in range(N_CORES):
        r0 = c * R
        # adj_r[p, g, ib, i] = adj[r0 + ib*IW + i, g*128 + p]
        blk = adj[r0:r0 + R, :]                          # [R, N]
        adj_r = np.ascontiguousarray(
            blk.reshape(IB, IW, G, 128).transpose(3, 2, 0, 1)
        ).astype(ml_dtypes.bfloat16)                     # [128, G, IB, IW]
        # g1b[p, k, ib, i] = G1[r0 + ib*IW + i, k]
        g1_blk = G1[r0:r0 + R, :].reshape(IB, IW, K).transpose(2, 0, 1)
        g1b = np.broadcast_to(
            g1_blk[None].astype(ml_dtypes.bfloat16), (128, K, IB, IW))
        g1b = np.ascontiguousarray(g1b)
        in_maps.append({
            "featT": featT,
            "wcat": wcat,
            "adjr": adj_r,
            "g1b": g1b,
            "g2t": g2t,
            "g2e": g2e,
            "e25t": e25t,
        })
    return in_maps


def kernel(features, adj, W, a):
    nc = build_program()
    in_maps = prepare_inputs(features, adj, W, a)
    res = run_bass_kernel_spmd(nc, in_maps, list(range(N_CORES)))
    out = np.concatenate(
        [res.results[c]["out"] for c in range(N_CORES)], axis=0)
    return out.astype(np.float32)


if __name__ == "__main__":
    rng = np.random.default_rng(0)
    features = rng.standard_normal((N, F_IN), dtype=np.float32)
    adj = (rng.integers(0, 2, size=(N, N))).astype(np.float32)
    W = (rng.standard_normal((K, F_IN, H), dtype=np.float32) * 0.118)
    a = (rng.standard_normal((K, 2 * H), dtype=np.float32) * 0.176)
    out = kernel(features=features, adj=adj, W=W, a=a)
    print("out", out.shape, out.dtype, np.abs(out).max())


# revision 27
# speedup vs baseline: 4.8656x; 1.0297x over previous
"""GAT layer (N=8192, F_IN=256, H=64 per head, K=8 heads) on 8 Trainium2 cores.

Strategy (row-sharding, fully data-parallel, no collectives):
  reference per head k:
    h   = features @ W[k]                      [N, H]
    wh1 = h @ a[k,:H]; wh2 = h @ a[k,H:]       [N]
    e   = leaky_relu(wh1[:,None] + wh2[None,:], 0.2)
    att = softmax(where(adj>0, e, -9e15), axis=1)
    out = elu(att @ h)

  Algebra: with s = wh1[i] + wh2[j],
    exp(lrelu(s)) = exp(0.2 s) * max(exp(0.8 s), 1)
                  = e1_{i} * e2_{j} * max(G1_i * G2_j, 1)
  where G1 = exp(0.8 wh1), G2 = exp(0.8 wh2), e1 = exp(0.2 wh1), e2 = exp(0.2 wh2).
  The row factor e1_i cancels in softmax.  The column factor e2_j is folded
  into the value matrix (or the score scalars).  adj in {0,1} multiplies
  exactly:
    unnorm[i,j] = adj[i,j] * max(G1_i G2_j, 1) * e2_j   (up to cancelled e1_i)
    out[i]      = elu( (unnorm @ h) / (unnorm @ 1) )

  Per (head k, col-group g) "unit" the device needs one score op and one
  mask multiply over a [128, 1024] tile, plus a PE matmul pair.  Units are
  scheduled onto engines fractionally per head to balance busy time:
    - score 'v' (DVE tensor_scalar mult+max, 4x mode) or
      score 'a' (ACT relu(G2*g1b - 1) = zm - 1, plus a correction matmul
      acc += h_sb @ adj on the PE)
    - mask multiply on DVE (2x mode) or Pool (GPSIMD)
  Heads that ever use the ACT path carry values [h*e2 | e2]; pure-DVE heads
  carry [h | 1] and fold e2 into the score scalars (s1=G2*e2, s2=e2).

Per-core layout ([j,i]-transposed tiles so contraction j sits on partitions):
  adj_r  [128, 64, 2, 512] bf16 : adj_r[p,g,ib,i] = adj[r0+ib*512+i, g*128+p]
  g1b    [128, 8, 2, 512] bf16  : G1 row broadcast across partitions
  g2t    [128, 8, 64] f32       : g2t[p,k,g] = G2[g*128+p, k]
  g2e    [128, 8, 64] f32       : (G2*E25) likewise (folded-e2 scalars)
  e25t   [128, 64, 8] f32       : e25t[p,g,k] = e2[g*128+p, k]
  featT  [256, 8192] bf16       : features.T (replicated; h computed on device)
  w_cat  [256, 512] bf16        : all heads' W side by side
"""

import sys
import os

sys.path.insert(0, "/opt/trn_rl_repo")

import numpy as np
import ml_dtypes
from contextlib import ExitStack

import concourse.bass as bass
import concourse.tile as tile
from concourse import bacc, mybir
from concourse.bass_utils import run_bass_kernel_spmd

N = 8192
F_IN = 256
H = 64
K = 8
ALPHA = 0.2
N_CORES = 8
R = N // N_CORES          # 1024 rows per core
IB = 2                    # i-blocks per core (512 columns of out-rows each)
IW = R // IB              # 512, i-width per block
G = N // 128              # 64 j-groups of 128
G_SUB = 8                 # j-groups per adjacency DMA
HA = H + 1                # 65: head value columns + denominator column

F32 = mybir.dt.float32
BF16 = mybir.dt.bfloat16
AX = mybir.AluOpType

_cached = {}

# fraction of each head's 64 column-groups whose score runs on ACT
Z_ACT_DEFAULT = (0.0, 0.0, 0.0, 0.0, 0.5, 1.0, 1.0, 1.0)
# fraction of each head's groups whose mask multiply runs on Pool/GPSIMD
M_POOL_DEFAULT = (1.0, 0.125, 1.0, 0.0, 0.0, 0.0, 0.0, 0.0)


def _spread(frac, n):
    """Evenly-spread boolean schedule of round(frac*n) True slots."""
    return [int((g + 1) * frac) - int(g * frac) > 0 for g in range(n)]


def build_program(loop_t=1, z_act=Z_ACT_DEFAULT, m_pool=M_POOL_DEFAULT,
                  sweeps=((0, 1, 5, 6), (2, 3, 4, 7)), pa_mode="dve",
                  drain_mode="act", lag=4):
    key = (loop_t, tuple(z_act), tuple(m_pool), tuple(map(tuple, sweeps)),
           pa_mode, drain_mode, lag)
    if key in _cached:
        return _cached[key]

    z_act_gs = [_spread(z_act[k], G) for k in range(K)]
    m_pool_gs = [_spread(m_pool[k], G) for k in range(K)]
    # heads that ever use the ACT score path carry [h*e2 | e2] values
    acty = [any(z_act_gs[k]) for k in range(K)]
    act_heads = [k for k in range(K) if acty[k]]
    oth_heads = [k for k in range(K) if not acty[k]]
    # slot layouts are built per contiguous range
    assert act_heads == list(range(min(act_heads), max(act_heads) + 1)) if \
        act_heads else True
    assert oth_heads == list(range(min(oth_heads), max(oth_heads) + 1)) if \
        oth_heads else True

    nc = bacc.Bacc("TRN2", target_bir_lowering=False, debug=False,
                   num_devices=N_CORES)

    featT_d = nc.dram_tensor("featT", [F_IN, N], BF16, kind="ExternalInput").ap()
    wcat_d = nc.dram_tensor("wcat", [F_IN, K * H], BF16, kind="ExternalInput").ap()
    adjr_d = nc.dram_tensor("adjr", [128, G, IB, IW], BF16, kind="ExternalInput").ap()
    g1b_d = nc.dram_tensor("g1b", [128, K, IB, IW], BF16, kind="ExternalInput").ap()
    g2t_d = nc.dram_tensor("g2t", [128, K, G], F32, kind="ExternalInput").ap()
    g2e_d = nc.dram_tensor("g2e", [128, K, G], F32, kind="ExternalInput").ap()
    e25t_d = nc.dram_tensor("e25t", [128, G, K], F32, kind="ExternalInput").ap()
    out_d = nc.dram_tensor("out", [R, K * H], F32, kind="ExternalOutput").ap()

    with tile.TileContext(nc) as tc:
        with ExitStack() as ctx:
            const = ctx.enter_context(tc.tile_pool(name="const", bufs=1))
            hpool = ctx.enter_context(tc.tile_pool(name="h", bufs=G))
            fpool = ctx.enter_context(tc.tile_pool(name="feat", bufs=4))
            adj_pool = ctx.enter_context(tc.tile_pool(name="adj", bufs=2))
            z_pool = ctx.enter_context(tc.tile_pool(name="z", bufs=6))
            u_pool = ctx.enter_context(tc.tile_pool(name="u", bufs=6))
            stg_pool = ctx.enter_context(tc.tile_pool(name="stg", bufs=4))
            fin_pool = ctx.enter_context(tc.tile_pool(name="fin", bufs=2))
            acc_sb_pool = ctx.enter_context(tc.tile_pool(name="accsb", bufs=2))
            psum = ctx.enter_context(tc.tile_pool(name="psum", bufs=8, space="PSUM"))

            # ---- constants ----
            g1b_sb = const.tile([128, K, IB, IW], BF16)
            nc.sync.dma_start(g1b_sb[:], g1b_d[:])
            g2t_sb = const.tile([128, K, G], F32)
            nc.sync.dma_start(g2t_sb[:], g2t_d[:])
            g2e_sb = const.tile([128, K, G], F32)
            nc.sync.dma_start(g2e_sb[:], g2e_d[:])
            e25t_sb = const.tile([128, G, K], F32)
            nc.sync.dma_start(e25t_sb[:], e25t_d[:])
            wcat_sb0 = const.tile([128, K * H], BF16)
            nc.sync.dma_start(wcat_sb0[:], wcat_d[0:128, :])
            wcat_sb1 = const.tile([128, K * H], BF16)
            nc.sync.dma_start(wcat_sb1[:], wcat_d[128:256, :])
            ident = const.tile([128, 128], F32)
            from concourse.masks import make_identity
            make_identity(nc, ident[:])
            negone = const.tile([128, 1], F32)
            nc.vector.memset(negone[:], -1.0)

            loop_cm = tc.For_i(0, loop_t, 1) if loop_t > 1 else None
            if loop_cm is not None:
                ctx.enter_context(loop_cm)

            # ---- phase A ----
            # act-ish heads: h_sb slot = [h*e2 | e2]; pure-v: [h | 1]
            n_act = len(act_heads)
            GF = 8                       # j-groups per featT DMA slab
            h_sb = []
            fslabs = {}
            for g in range(G):
                if g % GF == 0:
                    f0 = fpool.tile([128, GF * 128], BF16, tag="f")
                    nc.sync.dma_start(
                        f0[:], featT_d[0:128, g * 128:(g + GF) * 128])
                    f1 = fpool.tile([128, GF * 128], BF16, tag="f")
                    nc.sync.dma_start(
                        f1[:], featT_d[128:256, g * 128:(g + GF) * 128])
                    fslabs = (f0, f1)
                go = (g % GF) * 128
                ph = psum.tile([128, K * H], F32, tag="ps")
                nc.tensor.matmul(ph[:], fslabs[0][:, go:go + 128],
                                 wcat_sb0[:], start=True, stop=False)
                nc.tensor.matmul(ph[:], fslabs[1][:, go:go + 128],
                                 wcat_sb1[:], start=False, stop=True)

                hg = hpool.tile([128, K, HA], BF16, tag="h")
                phv = ph[:].rearrange("p (k f) -> p k f", k=K)
                n_oth = len(oth_heads)
                a0 = (act_heads[0] if n_act else 0)
                if n_oth:
                    o0 = oth_heads[0]
                    nc.scalar.copy(hg[:, o0:o0 + n_oth, 0:H],
                                   phv[:, o0:o0 + n_oth, :])
                    nc.gpsimd.memset(hg[:, o0:o0 + n_oth, H:HA], 1.0)
                if n_act:
                    e25b = (e25t_sb[:, g, a0:a0 + n_act]
                            .unsqueeze(2).broadcast_to((128, n_act, H)))
                    if pa_mode == "pool":
                        nc.gpsimd.tensor_tensor(
                            hg[:, a0:a0 + n_act, 0:H],
                            phv[:, a0:a0 + n_act, :], e25b, op=AX.mult)
                    else:
                        nc.vector.tensor_tensor(
                            hg[:, a0:a0 + n_act, 0:H],
                            phv[:, a0:a0 + n_act, :], e25b, op=AX.mult)
                    nc.gpsimd.tensor_copy(hg[:, a0:a0 + n_act, H:HA],
                                          e25t_sb[:, g, a0:a0 + n_act]
                                          .unsqueeze(2))
                h_sb.append(hg)

            # ---- phase B: two 4-head sweeps over the adjacency ----
            PAIR = IB * IW  # 1024

            for sweep_heads in sweeps:
                accs = {}
                for k in sweep_heads:
                    for ib in range(IB):
                        accs[(k, ib)] = psum.tile(
                            [HA, IW], F32, tag="ps", name=f"acc{k}_{ib}")
                for gs in range(G // G_SUB):
                    adj_t = adj_pool.tile([128, G_SUB, IB, IW], BF16)
                    nc.sync.dma_start(
                        adj_t[:], adjr_d[:, gs * G_SUB:(gs + 1) * G_SUB, :, :]
                    )
                    for gi in range(G_SUB):
                        g = gs * G_SUB + gi
                        adj_pair = adj_t[:, gi, :, :].rearrange("p b i -> p (b i)")
                        # two half-tiles (2 heads each) double pipeline depth
                        halves = [z_pool.tile([128, 2, PAIR], BF16, tag="z",
                                               name=f"zh{g}_{i2}")
                                  for i2 in range(2)]
                        u_halves = [u_pool.tile([128, 2, PAIR], BF16, tag="u",
                                                name=f"uh{g}_{i2}")
                                    for i2 in range(2)]
                        for s, k in enumerate(sweep_heads):
                            z_sl = halves[s // 2][:, s % 2, :]
                            g1b_pair = g1b_sb[:, k, :, :].rearrange(
                                "p b i -> p (b i)")
                            if z_act_gs[k][g]:
                                # z' = relu(G2*g1b - 1) = zm - 1 (+corr matmul)
                                nc.scalar.activation(
                                    z_sl, g1b_pair,
                                    mybir.ActivationFunctionType.Relu,
                                    bias=negone[:],
                                    scale=g2t_sb[:, k, g:g + 1],
                                )
                            elif acty[k]:
                                # values carry e2 -> plain zm = max(z*G2, 1)
                                nc.vector.tensor_scalar(
                                    z_sl, g1b_pair,
                                    g2t_sb[:, k, g:g + 1], 1.0,
                                    op0=AX.mult, op1=AX.max,
                                )
                            else:
                                # e2 folded into score: zm' = max(z*G2e2, e2)
                                nc.vector.tensor_scalar(
                                    z_sl, g1b_pair,
                                    g2e_sb[:, k, g:g + 1],
                                    e25t_sb[:, g, k:k + 1],
                                    op0=AX.mult, op1=AX.max,
                                )
                        # masks: batch contiguous DVE slots within each half
                        for h2 in range(2):
                            sl = [h2 * 2, h2 * 2 + 1]
                            pool_sl = [s for s in sl
                                       if m_pool_gs[sweep_heads[s]][g]]
                            dve_sl = [s for s in sl if s not in pool_sl]
                            for s in pool_sl:
                                nc.gpsimd.tensor_tensor(
                                    u_halves[h2][:, s % 2, :],
                                    halves[h2][:, s % 2, :], adj_pair,
                                    op=AX.mult)
                            if len(dve_sl) == 2:
                                nc.vector.tensor_tensor(
                                    u_halves[h2][:],
                                    halves[h2][:],
                                    adj_pair.unsqueeze(1)
                                    .broadcast_to((128, 2, PAIR)),
                                    op=AX.mult)
                            else:
                                for s in dve_sl:
                                    nc.vector.tensor_tensor(
                                        u_halves[h2][:, s % 2, :],
                                        halves[h2][:, s % 2, :], adj_pair,
                                        op=AX.mult)
                        for s, k in enumerate(sweep_heads):
                            is_act = z_act_gs[k][g]
                            u_sl = u_halves[s // 2]
                            for ib in range(IB):
                                nc.tensor.matmul(
                                    accs[(k, ib)][:],
                                    h_sb[g][:, k, :],
                                    u_sl[:, s % 2,
                                         ib * IW:(ib + 1) * IW],
                                    start=(g == 0),
                                    stop=(g == G - 1 and not is_act),
                                )
                            if is_act:  # correction: acc += h_sb @ adj
                                for ib in range(IB):
                                    nc.tensor.matmul(
                                        accs[(k, ib)][:],
                                        h_sb[g][:, k, :],
                                        adj_t[:, gi, ib, :],
                                        start=False,
                                        stop=(g == G - 1),
                                    )
                # drain this sweep: transpose, divide, elu, store
                sw_sorted = sorted(sweep_heads)
                runs = []
                for k in sw_sorted:
                    if runs and runs[-1][-1] == k - 1:
                        runs[-1].append(k)
                    else:
                        runs.append([k])
                for ib in range(IB):
                    stgs = [stg_pool.tile([128, 4, HA], F32, tag="stg",
                                          name=f"stg{sweep_heads[0]}_{ib}_{c}")
                            for c in range(IW // 128)]
                    for s, k in enumerate(sw_sorted):
                        acc_sb = acc_sb_pool.tile([HA, IW], F32, tag="accsb")
                        nc.scalar.copy(acc_sb[:], accs[(k, ib)][:])
                        for c in range(IW // 128):
                            pst = psum.tile([128, HA], F32, tag="ps",
                                            name=f"pst{k}_{ib}_{c}")
                            nc.tensor.transpose(
                                pst[:], acc_sb[:, c * 128:(c + 1) * 128],
                                ident[0:HA, 0:HA],
                            )
                            nc.scalar.copy(stgs[c][:, s, :], pst[:])
                    for c in range(IW // 128):
                        stg = stgs[c]
                        recips = fin_pool.tile([128, 4], F32, tag="recip")
                        nc.vector.reciprocal(recips[:], stg[:, :, H])
                        fin = fin_pool.tile([128, 4, H], F32, tag="fin")
                        nc.vector.tensor_tensor(
                            fin[:], stg[:, :, 0:H],
                            recips[:].unsqueeze(2).broadcast_to((128, 4, H)),
                            op=AX.mult,
                        )
                        # elu(x) = exp(min(x,0)) + (max(x,0) - 1)
                        fin2 = fin_pool.tile([128, 4 * H], F32, tag="fin2")
                        finf = fin[:].rearrange("p k f -> p (k f)")
                        nc.vector.tensor_scalar(
                            fin2[:], finf, 0.0, None, op0=AX.min
                        )
                        ex = fin_pool.tile([128, 4 * H], F32, tag="ex")
                        nc.scalar.activation(
                            ex[:], fin2[:], mybir.ActivationFunctionType.Exp
                        )
                        rel = fin_pool.tile([128, 4 * H], F32, tag="rel")
                        nc.vector.tensor_scalar(
                            rel[:], finf, 0.0, -1.0, op0=AX.max, op1=AX.add
                        )
                        res = fin_pool.tile([128, 4 * H], F32, tag="res")
                        nc.vector.tensor_tensor(res[:], ex[:], rel[:], op=AX.add)
                        resv = res[:].rearrange("p (k f) -> p k f", k=4)
                        for run in runs:
                            s0 = sw_sorted.index(run[0])
                            nc.sync.dma_start(
                                out_d[ib * IW + c * 128:
                                      ib * IW + (c + 1) * 128,
                                      run[0] * H:(run[-1] + 1) * H],
                                resv[:, s0:s0 + len(run), :],
                            )

    nc.compile()
    _cached[key] = nc
    return nc


def prepare_inputs(features, adj, W, a):
    """Host-side prep: tiny projections + per-core sharded/transposed layouts."""
    features = np.asarray(features, dtype=np.float32)
    adj = np.asarray(adj, dtype=np.float32)
    W = np.asarray(W, dtype=np.float32)
    a = np.asarray(a, dtype=np.float32)

    # av[k] = W[k] @ a[k]  -> wh = features @ av.T   (tiny: K*F_IN*H flops)
    av1 = np.einsum("kfh,kh->kf", W, a[:, :H])          # [K, F_IN]
    av2 = np.einsum("kfh,kh->kf", W, a[:, H:])          # [K, F_IN]
    wh1 = features @ av1.T                               # [N, K]
    wh2 = features @ av2.T                               # [N, K]
    G1 = np.exp(0.8 * wh1).astype(np.float32)            # row factors
    G2 = np.exp(0.8 * wh2).astype(np.float32)            # col factors
    E25 = np.exp(0.2 * wh2).astype(np.float32)           # folded into values

    featT = np.ascontiguousarray(features.T).astype(ml_dtypes.bfloat16)
    wcat = np.ascontiguousarray(
        W.transpose(1, 0, 2).reshape(F_IN, K * H)).astype(ml_dtypes.bfloat16)

    # g2t[p,k,g] = G2[g*128+p, k];  e25t[p,g,k] = E25[g*128+p, k]
    g2t = np.ascontiguousarray(
        G2.reshape(G, 128, K).transpose(1, 2, 0))        # [128, K, G]
    g2e = np.ascontiguousarray(
        (G2 * E25).reshape(G, 128, K).transpose(1, 2, 0))  # [128, K, G]
    e25t = np.ascontiguousarray(
        E25.reshape(G, 128, K).transpose(1, 0, 2))       # [128, G, K]

    in_maps = []
    for c in range(N_CORES):
        r0 = c * R
        # adj_r[p, g, ib, i] = adj[r0 + ib*IW + i, g*128 + p]
        blk = adj[r0:r0 + R, :]                          # [R, N]
        adj_r = np.ascontiguousarray(
            blk.reshape(IB, IW, G, 128).transpose(3, 2, 0, 1)
        ).astype(ml_dtypes.bfloat16)                     # [128, G, IB, IW]
        # g1b[p, k, ib, i] = G1[r0 + ib*IW + i, k]
        g1_blk = G1[r0:r0 + R, :].reshape(IB, IW, K).transpose(2, 0, 1)
        g1b = np.broadcast_to(
            g1_blk[None].astype(ml_dtypes.bfloat16), (128, K, IB, IW))
        g1b = np.ascontiguousarray(g1b)
        in_maps.append({
            "featT": featT,
            "wcat": wcat,
            "adjr": adj_r,
            "g1b": g1b,
            "g2t": g2t,
            "g2e": g2e,
            "e25t": e25t,
        })
    return in_maps


def kernel(features, adj, W, a):
    nc = build_program()
    in_maps = prepare_inputs(features, adj, W, a)
    res = run_bass_kernel_spmd(nc, in_maps, list(range(N_CORES)))
    out = np.concatenate(
        [res.results[c]["out"] for c in range(N_CORES)], axis=0)
    return out.astype(np.float32)


if __name__ == "__main__":
    rng = np.random.default_rng(0)
    features = rng.standard_normal((N, F_IN), dtype=np.float32)
    adj = (rng.integers(0, 2, size=(N, N))).astype(np.float32)
    W = (rng.standard_normal((K, F_IN, H), dtype=np.float32) * 0.118)
    a = (rng.standard_normal((K, 2 * H), dtype=np.float32) * 0.176)
    out = kernel(features=features, adj=adj, W=W, a=a)
    print("out", out.shape, out.dtype, np.abs(out).max())
